# revision 56
# baseline (speedup 1.0000x reference)
"""CapsNet forward kernel for Trainium2, 8-core data-parallel.

Strategy (per spec sharding_hint): batch (512) split across 8 cores (64 each);
all params replicated. Routing logits b are a batch-mean -> AllReduce of
per-core partial deltas (1152 floats) per routing round (rounds 1,2 only;
round 3's b update is dead in the reference).

v4: all big matmuls fp16 (1 cycle/row); host-side im2col for conv1 and
host-retiled weights so every weight DMA is contiguous (128 descriptors,
not ~10k); routing contraction tiled by (co_blk, pix) so it consumes the
primary-caps output directly from SBUF (no DRAM round-trip / scatter DMAs);
routing logits kept as [32, 36] (n = c32*36 + pix) with tiny mask-matmuls
for the cross-partition regroups; one AllReduce per round (rounds 1-2) and
a warm-up collective mid-conv to absorb inter-core skew.

Math (keeps exact semantics, never materializes u):
  n = c32*36 + pix, co = s*32 + c32, r = s*1152 + n
  xr[co, pix, b]   primary-caps relu output (SBUF, 2 co-halves)
  W2[co, pix, hl]  = W.transpose(3,0,1,2).reshape(9216,160) re-indexed
  s[b,hl]  = sum_{co,pix} c[n] * xr * W2          (72 PE matmuls, K=128)
  v        = squash_dim1(s)
  P[co,pix,b] = sum_hl W2 * v[b,hl]               (PE)
  delta[n] = 1/(B*160) * sum_{s,b} xr * P         (DVE + mask matmul)
Convs are PE matmuls: conv1 via host-im2col patches (K=81), primary-caps
conv via 81 shifted-window matmuls accumulated in PSUM (K=256 as 2x128).
"""

import numpy as np

import concourse.bass as bass
import concourse.mybir as mybir
import concourse.tile as tile
from concourse.ap import AP
from concourse.bass_utils import run_bass_kernel_spmd

F32 = mybir.dt.float32
F16 = mybir.dt.float16
AL = mybir.AluOpType
AF = mybir.ActivationFunctionType
AX = mybir.AxisListType

NCORES = 8
B = 512
BC = B // NCORES           # 64 images per core
MAX_WAITS = 1              # walrus on this path allows 1 sync wait per inst
HL = 160                   # 10 classes x 16 pose
GROUPS = [(0, 14), (14, 14), (28, 14), (42, 14), (56, 8)]  # conv2 image groups
ROUTE_SCALE = 1.0 / (B * HL)
CHUNK = 8                  # conv1 images per im2col DMA chunk


def _r(t, dims):
    """Raw AP on tile/ap t with explicit [step, count] dims (elements)."""
    return AP(t.tensor, t.offset, dims)


def split_waits(nc, max_waits=MAX_WAITS):
    """This walrus build rejects >max_waits sync waits per instruction; move
    excess waits onto same-engine NoOps inserted immediately before."""
    for f in nc.m.functions:
        for blk in f.blocks:
            out = []
            for ins in blk.instructions:
                si = ins.sync_info
                if si is not None and si.on_wait and len(si.on_wait) > max_waits:
                    waits = list(si.on_wait)
                    k = 0
                    while len(waits) > max_waits:
                        chunk, waits = waits[:max_waits], waits[max_waits:]
                        nop = mybir.InstNoOp(name=f"{ins.name}-ws{k}", ins=[], outs=[])
                        nop.engine = ins.engine
                        nop.sync_info = mybir.SyncInfo(on_wait=chunk, on_update=[])
                        out.append(nop)
                        k += 1
                    ins.sync_info = mybir.SyncInfo(
                        on_wait=waits, on_update=list(si.on_update or []))
                out.append(ins)
            blk.instructions = out


def build_nc():
    nc = bass.Bass(num_devices=NCORES)

    xpatch = nc.dram_tensor("xpatch", [81, BC * 560], F16, kind="ExternalInput")
    w1t = nc.dram_tensor("w1t", [81, 256], F16, kind="ExternalInput")
    b1 = nc.dram_tensor("b1", [256], F32, kind="ExternalInput")
    # conv2 weights, retiled: [co_blk, ci_blk, 128ci, 81kk * 128co]
    pcw4 = nc.dram_tensor("pcw4", [2, 2, 128, 81 * 128], F16,
                          kind="ExternalInput")
    pcb = nc.dram_tensor("pcb", [256], F32, kind="ExternalInput")
    # routing weights in (co_blk, pix) tiling
    w2p = nc.dram_tensor("w2p", [2, 128, 36 * HL], F16, kind="ExternalInput")
    w2pt_a = nc.dram_tensor("w2pt_a", [2, 128, 36 * 128], F16,
                            kind="ExternalInput")
    w2pt_b = nc.dram_tensor("w2pt_b", [2, 32, 36 * 128], F16,
                            kind="ExternalInput")
    eye64 = nc.dram_tensor("eye64", [BC, BC], F16, kind="ExternalInput")
    maskT = nc.dram_tensor("maskT", [128, 32], F32, kind="ExternalInput")
    mask32 = nc.dram_tensor("mask32", [32, 128], F32, kind="ExternalInput")
    vout = nc.dram_tensor("vout", [BC, HL], F32, kind="ExternalOutput")

    with tile.TileContext(nc) as tc:
        with (
            tc.tile_pool(name="pers", bufs=1) as pers,
            tc.tile_pool(name="dram", bufs=1, space="DRAM") as dpool,
            tc.tile_pool(name="sps", bufs=1, space="PSUM") as sps,
        ):
            w1t_sb = pers.tile([81, 256], F16)
            nc.sync.dma_start(w1t_sb[:], w1t[:])
            b1_sb = pers.tile([128, 2], F32)
            nc.sync.dma_start(b1_sb[:], _r(b1[:], [[1, 128], [128, 2]]))
            pcb_sb = pers.tile([128, 2], F32)
            nc.sync.dma_start(pcb_sb[:], _r(pcb[:], [[1, 128], [128, 2]]))
            ones32 = pers.tile([32, 1], F32)
            nc.gpsimd.memset(ones32[:], 1.0)
            ones1 = pers.tile([1, 32], F32)
            nc.gpsimd.memset(ones1[:], 1.0)
            b32 = pers.tile([32, 36], F32)
            eye_sb = pers.tile([BC, BC], F16)
            maskT_sb = pers.tile([128, 32], F32)
            mask32_sb = pers.tile([32, 128], F32)
            # routing s-weights [co_blk][128, 36*160]; prefetched during conv1
            w2p_sb = [pers.tile([128, 36 * HL], F16, tag=f"w2p{cb}",
                                name=f"w2p{cb}") for cb in range(2)]
            # xr (primary caps output), written by conv2 epilogue
            xr_sb = [pers.tile([128, 36 * BC], F16, tag=f"xr{cb}",
                               name=f"xr{cb}") for cb in range(2)]

            # ---------------- conv phase ----------------
            with (
                tc.tile_pool(name="h1p", bufs=1) as h1p,
                tc.tile_pool(name="w2cp", bufs=2) as w2cp,
            ):
                h1s = [h1p.tile([128, BC * 400], F16, tag=f"h1_{ci}",
                                name=f"h1_{ci}")
                       for ci in range(2)]

                def load_w2c(co, ci, eng):
                    t = w2cp.tile([128, 81 * 128], F16, tag="w2c")
                    eng.dma_start(
                        t[:],
                        AP(pcw4[:].tensor, (co * 2 + ci) * 128 * 81 * 128,
                           [[81 * 128, 128], [1, 81 * 128]]),
                    )
                    return t

                with (
                    tc.tile_pool(name="pwp", bufs=3) as pwp,
                    tc.tile_pool(name="ps1p", bufs=2, space="PSUM") as ps1p,
                    tc.tile_pool(name="ps2p", bufs=1, space="PSUM") as ps2p,
                ):
                    NCH = BC // CHUNK
                    pas = {}
                    n_warm = 0

                    def load_chunk(k, eng):
                        pa = pwp.tile([81, CHUNK * 560], F16, tag="pa")
                        eng.dma_start(
                            pa[:],
                            AP(xpatch[:].tensor, k * CHUNK * 560,
                               [[BC * 560, 81], [1, CHUNK * 560]]),
                        )
                        pas[k] = pa

                    def conv1_pair(k, lp):
                        """conv1 for an image pair: 2 matmuls into one
                        bank-padded PSUM tile per ci, then ONE batched relu
                        per (pair, ci), Scalar / DVE split."""
                        pa = pas[k]
                        pstep = pa.ap[0][0]
                        gi = k * CHUNK + lp * 2
                        for ci in range(2):
                            ps = ps1p.tile([128, 1024], F32, tag="ps1")
                            pp = ps.ap[0][0]
                            for u in range(2):
                                rhs = AP(pa.tensor,
                                         pa.offset + (lp * 2 + u) * 560,
                                         [[pstep, 81], [28, 20], [1, 20]])
                                nc.tensor.matmul(
                                    AP(ps.tensor, ps.offset + u * 512,
                                       [[pp, 128], [1, 400]]),
                                    w1t_sb[:, ci * 128:(ci + 1) * 128],
                                    rhs,
                                    start=True, stop=True,
                                )
                            dst = AP(h1s[ci].tensor,
                                     h1s[ci].offset + gi * 400,
                                     [[h1s[ci].ap[0][0], 128],
                                      [400, 2], [1, 400]])
                            src = _r(ps, [[pp, 128], [512, 2], [1, 400]])
                            if ci == 0:
                                nc.scalar.activation(
                                    dst, src, AF.Relu,
                                    bias=b1_sb[:, ci:ci + 1],
                                )
                            else:
                                nc.vector.tensor_scalar(
                                    dst, src, b1_sb[:, ci:ci + 1], 0.0,
                                    AL.add, AL.max,
                                )

                    def conv1_chunk(k):
                        for lp in range(CHUNK // 2):
                            conv1_pair(k, lp)
                        pas.pop(k)

                    def conv2_group(co_blk, g, g0, nb, w2cs, interleave=()):
                        """One conv2 output group; optionally interleaves
                        pending conv1 pair thunks into the matmul stream so
                        their epilogues drain during conv2's long PSUM
                        accumulation (no conv1 stalls)."""
                        interleave = list(interleave)
                        stride = max(1, (162 // (len(interleave) + 1))
                                     if interleave else 162)
                        ps2 = ps2p.tile([128, 504], F32, tag="ps2", bufs=2)
                        pstep = ps2.ap[0][0]
                        out4 = _r(ps2, [[pstep, 128], [36, nb], [6, 6], [1, 6]])
                        n_mm = 0
                        for ci in range(2):
                            h1 = h1s[ci]
                            hp = h1.ap[0][0]
                            for kk in range(81):
                                ky, kx = divmod(kk, 9)
                                rhs = AP(h1.tensor,
                                         h1.offset + g0 * 400 + ky * 20 + kx,
                                         [[hp, 128], [400, nb], [40, 6], [2, 6]])
                                nc.tensor.matmul(
                                    out4,
                                    w2cs[ci][:, kk * 128:(kk + 1) * 128],
                                    rhs,
                                    start=(ci == 0 and kk == 0),
                                    stop=(ci == 1 and kk == 80),
                                )
                                n_mm += 1
                                if interleave and n_mm % stride == 0:
                                    interleave.pop(0)()
                        for t in interleave:
                            t()
                        # bias+relu, write pix-major (col = pix*BC + b)
                        xr = xr_sb[co_blk]
                        nc.scalar.activation(
                            AP(xr.tensor, xr.offset + g0,
                               [[xr.ap[0][0], 128], [1, nb], [BC, 36]]),
                            _r(ps2, [[pstep, 128], [36, nb], [1, 36]]),
                            AF.Relu,
                            bias=pcb_sb[:, co_blk:co_blk + 1],
                        )

                    def warmup_cc(dep_ap):
                        """Collective keyed on conv progress: absorbs
                        inter-core skew off the critical path so the real
                        AllReduces are fast."""
                        nonlocal n_warm
                        cinw = dpool.tile([128, 1], F32, name=f"cinw{n_warm}")
                        coutw = dpool.tile([128, 1], F32, name=f"coutw{n_warm}",
                                           addr_space="Shared")
                        n_warm += 1
                        nc.gpsimd.dma_start(cinw[:], dep_ap)
                        nc.gpsimd.collective_compute(
                            "AllReduce", AL.add,
                            replica_groups=[list(range(NCORES))],
                            ins=[cinw.opt()], outs=[coutw.opt()],
                        )

                    # group-pipelined conv: conv1 chunks feed conv2 co0
                    # groups, with later chunks' image-pairs interleaved INTO
                    # the conv2 matmul stream. ALL bulk DMAs ride the sync
                    # queue so the scalar engine stays free for epilogues.
                    load_chunk(0, nc.sync)
                    load_chunk(1, nc.sync)
                    w2c0 = [load_w2c(0, 0, nc.sync)]
                    load_chunk(2, nc.sync)
                    conv1_chunk(0)
                    load_chunk(3, nc.sync)
                    conv1_chunk(1)
                    w2c0.append(load_w2c(0, 1, nc.sync))
                    # small routing constants, nothing urgent
                    nc.sync.dma_start(eye_sb[:], eye64[:])
                    nc.sync.dma_start(maskT_sb[:], maskT[:])
                    nc.sync.dma_start(mask32_sb[:], mask32[:])
                    for cb in range(2):
                        nc.sync.dma_start(
                            w2p_sb[cb][:],
                            AP(w2p[:].tensor, cb * 128 * 36 * HL,
                               [[36 * HL, 128], [1, 36 * HL]]),
                        )
                    load_chunk(4, nc.sync)

                    def pairs(k, then_load=None):
                        out = []
                        for lp in range(CHUNK // 2):
                            def thunk(k=k, lp=lp, last=(lp == CHUNK // 2 - 1)):
                                conv1_pair(k, lp)
                                if last:
                                    pas.pop(k)
                                    if then_load is not None:
                                        load_chunk(then_load, nc.sync)
                            out.append(thunk)
                        return out

                    ilv = {0: pairs(2, 5) + pairs(3, 6),
                           1: pairs(4, 7) + pairs(5),
                           2: pairs(6), 3: pairs(7), 4: []}
                    for g, (g0, nb) in enumerate(GROUPS):
                        conv2_group(0, g, g0, nb, w2c0, ilv[g])
                        if g == 0:
                            warmup_cc(xr_sb[0][:, 0:1])
                    # round-1 s-matmul, co0 half: c is uniform in round 1, so
                    # these fold into the conv stream as soon as xr0 is done
                    s_ps1 = sps.tile([BC, HL], F32, tag="s_ps")
                    for pix in range(36):
                        nc.tensor.matmul(
                            s_ps1[:],
                            xr_sb[0][:, pix * BC:(pix + 1) * BC],
                            w2p_sb[0][:, pix * HL:(pix + 1) * HL],
                            start=(pix == 0), stop=False,
                        )
                    w2c1 = [load_w2c(1, 0, nc.sync), load_w2c(1, 1, nc.scalar)]
                    for g, (g0, nb) in enumerate(GROUPS):
                        conv2_group(1, g, g0, nb, w2c1)
                        if g == 0:
                            warmup_cc(xr_sb[1][:, 0:1])

            # ---------------- routing phase ----------------
            with (
                tc.tile_pool(name="rsb", bufs=1) as rsb,
                tc.tile_pool(name="rnd", bufs=2) as rnd,
                tc.tile_pool(name="gps", bufs=4, space="PSUM") as gps,
                tc.tile_pool(name="zps", bufs=1, space="PSUM") as zps,
            ):
                # W2^T for the P matmuls: loaded at routing start (h1 freed)
                w2pt_a_sb = [rsb.tile([128, 36 * 128], F16, tag=f"w2pta{cb}",
                                      name=f"w2pta{cb}") for cb in range(2)]
                for cb in range(2):
                    (nc.sync if cb == 0 else nc.scalar).dma_start(
                        w2pt_a_sb[cb][:],
                        AP(w2pt_a[:].tensor, cb * 128 * 36 * 128,
                           [[36 * 128, 128], [1, 36 * 128]]),
                    )
                w2pt_b_sb = [rsb.tile([32, 36 * 128], F16, tag=f"w2ptb{cb}",
                                      name=f"w2ptb{cb}") for cb in range(2)]
                for cb in range(2):
                    (nc.sync if cb == 0 else nc.scalar).dma_start(
                        w2pt_b_sb[cb][:],
                        AP(w2pt_b[:].tensor, cb * 32 * 36 * 128,
                           [[36 * 128, 32], [1, 36 * 128]]),
                    )
                prod = rsb.tile([128, 2 * 36 * BC], F16)

                def s_matmul():
                    s_ps = sps.tile([BC, HL], F32, tag="s_ps")
                    first, last = (0, 0), (1, 35)
                    for cb in range(2):
                        for pix in range(36):
                            nc.tensor.matmul(
                                s_ps[:],
                                xr_sb[cb][:, pix * BC:(pix + 1) * BC],
                                w2p_sb[cb][:, pix * HL:(pix + 1) * HL],
                                start=((cb, pix) == first),
                                stop=((cb, pix) == last),
                            )
                    return s_ps

                def squash(s_sb, out_dtype):
                    sq = rnd.tile([BC, HL], F32, tag="sq")
                    nc.scalar.square(sq[:], s_sb[:])
                    n2 = rnd.tile([BC, 16], F32, tag="n2")
                    nc.vector.tensor_reduce(
                        n2[:].rearrange("a b -> a b ()"),
                        _r(sq, [[sq.ap[0][0], BC], [1, 16], [16, 10]]),
                        AX.X, AL.add,
                    )
                    rt = rnd.tile([BC, 16], F32, tag="rt")
                    nc.scalar.sqrt(rt[:], n2[:])
                    n2p1 = rnd.tile([BC, 16], F32, tag="n2p1")
                    nc.vector.tensor_scalar_add(n2p1[:], n2[:], 1.0)
                    rcp = rnd.tile([BC, 16], F32, tag="rcp")
                    nc.vector.reciprocal(rcp[:], n2p1[:])
                    f = rnd.tile([BC, 16], F32, tag="f")
                    nc.vector.tensor_tensor(f[:], rt[:], rcp[:], AL.mult)
                    v_sb = rnd.tile([BC, HL], out_dtype, tag="v_sb")
                    nc.vector.tensor_tensor(
                        _r(v_sb, [[v_sb.ap[0][0], BC], [16, 10], [1, 16]]),
                        _r(s_sb, [[s_sb.ap[0][0], BC], [16, 10], [1, 16]]),
                        _r(f, [[f.ap[0][0], BC], [0, 10], [1, 16]]),
                        AL.mult,
                    )
                    return v_sb

                def p_delta_update(v16, rnd_idx, rce32):
                    """delta via P[co,pix,b] = sum_hl W2*v (PE), then
                    D[co,pix] = sum_b xr*P (DVE), then delta32[c32,pix] =
                    mask-matmul partition regroup. If xr is c-scaled, divide
                    by ce32 (rce32 ap) to undo."""
                    vt_ps = gps.tile([128, 2 * BC], F16, tag="vt_ps", bufs=1)
                    nc.tensor.transpose(vt_ps[:, 0:BC], v16[:, 0:128], eye_sb[:])
                    nc.tensor.transpose(
                        AP(vt_ps.tensor, vt_ps.offset + BC,
                           [[vt_ps.ap[0][0], 32], [1, BC]]),
                        v16[:, 128:160], eye_sb[:])
                    vt_a = rnd.tile([128, BC], F16, tag="vt_a")
                    nc.scalar.copy(vt_a[:], vt_ps[:, 0:BC])
                    vt_b = rnd.tile([32, BC], F16, tag="vt_b")
                    nc.scalar.copy(
                        vt_b[:],
                        AP(vt_ps.tensor, vt_ps.offset + BC,
                           [[vt_ps.ap[0][0], 32], [1, BC]]))
                    # P in 4-pix batches; DVE multiplies straight out of PSUM
                    TB = 4
                    for cb in range(2):
                        for pb in range(36 // TB):
                            p_ps = gps.tile([128, TB * BC], F32, tag="p_ps",
                                            bufs=2)
                            for j in range(TB):
                                pix = pb * TB + j
                                nc.tensor.matmul(
                                    p_ps[:, j * BC:(j + 1) * BC],
                                    w2pt_a_sb[cb][:, pix * 128:(pix + 1) * 128],
                                    vt_a[:],
                                    start=True, stop=False,
                                )
                                nc.tensor.matmul(
                                    p_ps[:, j * BC:(j + 1) * BC],
                                    w2pt_b_sb[cb][:, pix * 128:(pix + 1) * 128],
                                    vt_b[:],
                                    start=False, stop=True,
                                )
                            xh = xr_sb[cb]
                            nc.vector.tensor_tensor(
                                prod[:, (cb * 36 + pb * TB) * BC:
                                     (cb * 36 + pb * TB + TB) * BC],
                                AP(xh.tensor, xh.offset + pb * TB * BC,
                                   [[xh.ap[0][0], 128], [1, TB * BC]]),
                                p_ps[:],
                                AL.mult,
                            )
                    ds_ps = gps.tile([32, 36], F32, tag="ds_ps", bufs=1)
                    for cb in range(2):
                        D = rnd.tile([128, 36], F32, tag=f"D{cb}")
                        nc.vector.tensor_reduce(
                            D[:].rearrange("a b -> a b ()"),
                            AP(prod.tensor, prod.offset + cb * 36 * BC,
                               [[prod.ap[0][0], 128], [BC, 36], [1, BC]]),
                            AX.X, AL.add,
                        )
                        # regroup: delta32[c32,pix] = sum_{p: p%32==c32} D[p,pix]
                        nc.tensor.matmul(
                            ds_ps[:], maskT_sb[:], D[:],
                            start=(cb == 0), stop=(cb == 1),
                        )
                    delta32 = rnd.tile([32, 36], F32, tag="delta32")
                    if rce32 is not None:
                        nc.vector.tensor_tensor(
                            delta32[:], ds_ps[:], rce32[:], AL.mult)
                    else:
                        nc.scalar.copy(delta32[:], ds_ps[:])
                    cin = dpool.tile([32, 36], F32, name=f"cin{rnd_idx}")
                    cout = dpool.tile([32, 36], F32, name=f"cout{rnd_idx}",
                                      addr_space="Shared")
                    nc.gpsimd.dma_start(cin[:], delta32[:])
                    nc.gpsimd.collective_compute(
                        "AllReduce", AL.add,
                        replica_groups=[list(range(NCORES))],
                        ins=[cin.opt()], outs=[cout.opt()],
                    )
                    dsum = rnd.tile([32, 36], F32, tag="dsum")
                    nc.gpsimd.dma_start(dsum[:], cout[:])
                    if rnd_idx == 0:
                        nc.scalar.mul(b32[:], dsum[:], ROUTE_SCALE)
                    else:
                        sc = rnd.tile([32, 36], F32, tag="sc")
                        nc.scalar.mul(sc[:], dsum[:], ROUTE_SCALE)
                        nc.vector.tensor_tensor(b32[:], b32[:], sc[:], AL.add)

                def softmax_ce():
                    """ce32[c32,pix] = softmax(b32)[n=c32*36+pix], F32."""
                    e32 = rnd.tile([32, 36], F32, tag="e32")
                    nc.scalar.activation(e32[:], b32[:], AF.Exp)
                    rs = rnd.tile([32, 1], F32, tag="rs")
                    nc.vector.tensor_reduce(
                        rs[:].rearrange("a b -> a b ()"), e32[:], AX.X, AL.add)
                    z_ps = zps.tile([1, 1], F32, tag="z_ps")
                    nc.tensor.matmul(z_ps[:], ones32[:], rs[:], start=True, stop=True)
                    z_sb = rnd.tile([1, 1], F32, tag="z_sb")
                    nc.scalar.copy(z_sb[:], z_ps[:])
                    zb_ps = zps.tile([32, 1], F32, tag="zb_ps")
                    nc.tensor.matmul(zb_ps[:], ones1[:], z_sb[:], start=True, stop=True)
                    rz = rnd.tile([32, 1], F32, tag="rz")
                    nc.vector.reciprocal(rz[:], zb_ps[:])
                    ce32 = rnd.tile([32, 36], F32, tag="ce32")
                    nc.vector.tensor_scalar_mul(ce32[:], e32[:], rz[:])
                    return ce32

                def scale_xr(m32f32):
                    """xr[co, pix, b] *= m32[co%32, pix] in place."""
                    cm_ps = zps.tile([128, 36], F32, tag="cm_ps")
                    nc.tensor.matmul(cm_ps[:], mask32_sb[:], m32f32[:],
                                     start=True, stop=True)
                    for cb in range(2):
                        xh = xr_sb[cb]
                        nc.vector.tensor_tensor(
                            _r(xh, [[xh.ap[0][0], 128], [BC, 36], [1, BC]]),
                            _r(xh, [[xh.ap[0][0], 128], [BC, 36], [1, BC]]),
                            _r(cm_ps, [[cm_ps.ap[0][0], 128], [1, 36], [0, BC]]),
                            AL.mult,
                        )

                # ---- round 1 (c uniform; xr unscaled) ----
                # co0 half was accumulated into s_ps1 during the conv phase
                for pix in range(36):
                    nc.tensor.matmul(
                        s_ps1[:],
                        xr_sb[1][:, pix * BC:(pix + 1) * BC],
                        w2p_sb[1][:, pix * HL:(pix + 1) * HL],
                        start=False, stop=(pix == 35),
                    )
                s_sb = rnd.tile([BC, HL], F32, tag="s_sb")
                nc.scalar.mul(s_sb[:], s_ps1[:], 1.0 / 1152.0)
                v16 = squash(s_sb, F16)
                p_delta_update(v16, 0, None)
                # ---- round 2 ----
                ce2 = softmax_ce()
                scale_xr(ce2)
                s_ps = s_matmul()
                # off the critical path: runs on DVE while the PE streams s
                rce32 = rnd.tile([32, 36], F32, tag="rce32")
                nc.vector.reciprocal(rce32[:], ce2[:])
                s_sb = rnd.tile([BC, HL], F32, tag="s_sb")
                nc.scalar.copy(s_sb[:], s_ps[:])
                v16 = squash(s_sb, F16)
                p_delta_update(v16, 1, rce32)
                # ---- round 3 (b update dead) ----
                ce3 = softmax_ce()
                ratio32 = rnd.tile([32, 36], F32, tag="ratio32")
                nc.vector.tensor_tensor(ratio32[:], ce3[:], rce32[:], AL.mult)
                scale_xr(ratio32)
                s_ps = s_matmul()
                s_sb = rnd.tile([BC, HL], F32, tag="s_sb")
                nc.scalar.copy(s_sb[:], s_ps[:])
                v_sb = squash(s_sb, F32)
                nc.sync.dma_start(vout[:], v_sb[:])

    return nc


_NC_CACHE = None


def _get_nc():
    global _NC_CACHE
    if _NC_CACHE is None:
        nc = build_nc()
        split_waits(nc)
        _NC_CACHE = nc
    return _NC_CACHE


def prepare_inputs(x, conv1_w, conv1_b, pc_w, pc_b, W):
    x = np.asarray(x, np.float32)
    xf = np.zeros((B, 800), np.float16)
    xf[:, :784] = x.reshape(B, 784).astype(np.float16)
    # host-side im2col ("wide patch"): xp[i, (ky,kx), j] = xf[i, 28*ky+kx+j]
    xp = np.lib.stride_tricks.as_strided(
        xf, shape=(B, 9, 9, 560), strides=(1600, 56, 2, 2)).reshape(B, 81, 560)
    w1t = np.ascontiguousarray(
        np.asarray(conv1_w, np.float32).reshape(256, 81).T).astype(np.float16)
    b1 = np.ascontiguousarray(np.asarray(conv1_b, np.float32))
    # pcw4[co_blk, ci_blk, ci128, kk*128co] = pc_w[co, ci, ky, kx]
    pcw = np.asarray(pc_w, np.float32).reshape(256, 256, 81)  # [co, ci, kk]
    pcw4 = np.ascontiguousarray(
        pcw.reshape(2, 128, 2, 128, 81).transpose(0, 2, 3, 4, 1)
    ).astype(np.float16)  # [co_blk, ci_blk, ci128, kk, co128]
    pcb = np.ascontiguousarray(np.asarray(pc_b, np.float32).reshape(256))
    # W2cp[co, pix, hl] = W2n[co*36+pix, hl]
    w2n = np.asarray(W, np.float32).transpose(3, 0, 1, 2).reshape(9216, HL)
    w2cp = w2n.reshape(256, 36, HL)
    w2p = np.ascontiguousarray(
        w2cp.reshape(2, 128, 36 * HL)).astype(np.float16)
    w2t = w2cp.transpose(2, 1, 0)                 # [hl, pix, co]
    w2pt_a = np.ascontiguousarray(
        w2t[:128].reshape(128, 36, 2, 128).transpose(2, 0, 1, 3)
        .reshape(2, 128, 36 * 128)).astype(np.float16)
    w2pt_b = np.ascontiguousarray(
        w2t[128:].reshape(32, 36, 2, 128).transpose(2, 0, 1, 3)
        .reshape(2, 32, 36 * 128)).astype(np.float16)
    eye64 = np.eye(BC, dtype=np.float16)
    maskT = np.zeros((128, 32), np.float32)
    maskT[np.arange(128), np.arange(128) % 32] = 1.0
    mask32 = np.ascontiguousarray(maskT.T)
    in_maps = []
    for c in range(NCORES):
        in_maps.append({
            "xpatch": np.ascontiguousarray(
                xp[c * BC:(c + 1) * BC].transpose(1, 0, 2).reshape(81, BC * 560)),
            "w1t": w1t, "b1": b1, "pcw4": pcw4, "pcb": pcb, "w2p": w2p,
            "w2pt_a": w2pt_a, "w2pt_b": w2pt_b, "eye64": eye64,
            "maskT": maskT, "mask32": mask32,
        })
    return in_maps


def kernel(x, conv1_w, conv1_b, pc_w, pc_b, W, _trace=False, _trace_kwargs=None):
    nc = _get_nc()
    in_maps = prepare_inputs(x, conv1_w, conv1_b, pc_w, pc_b, W)
    res = run_bass_kernel_spmd(
        nc, in_maps, list(range(NCORES)),
        trace=_trace, **(_trace_kwargs or {}),
    )
    v = np.concatenate([np.asarray(res.results[c]["vout"]) for c in range(NCORES)], 0)
    out = v.reshape(B, 1, 1, 10, 16).astype(np.float32)
    if _trace:
        return out, res
    return out


# revision 57
# speedup vs baseline: 1.0151x; 1.0151x over previous
"""CapsNet forward kernel for Trainium2, 8-core data-parallel.

Strategy (per spec sharding_hint): batch (512) split across 8 cores (64 each);
all params replicated. Routing logits b are a batch-mean -> AllReduce of
per-core partial deltas (1152 floats) per routing round (rounds 1,2 only;
round 3's b update is dead in the reference).

Optimizations (1156us baseline -> ~695us):
- all big matmuls fp16 (1 cycle/row on the PE vs 2 for fp32-HIGH mode)
- host-side im2col for conv1; host-retiled weights so every weight DMA is
  contiguous (128 descriptors instead of ~10k strided ones)
- conv1 fully interleaved into the conv2 matmul stream (image-pair matmuls
  + batched Scalar/DVE relu epilogues between conv2 groups)
- routing contraction tiled by (co_blk, pix) so it consumes the primary-caps
  output directly from SBUF (no DRAM round-trip / scatter DMAs); routing
  logits kept as [32, 36] (n = c32*36 + pix) with tiny mask-matmuls for the
  cross-partition regroups
- round-1 s-matmul co0 half folded into the conv phase (c is uniform)
- one AllReduce per round (rounds 1-2), warm-up collectives during conv to
  absorb cold-start + inter-core skew
- P-phase products consumed by DVE straight out of PSUM (no staging copies)

Math (keeps exact semantics, never materializes u):
  n = c32*36 + pix, co = s*32 + c32, r = s*1152 + n
  xr[co, pix, b]   primary-caps relu output (SBUF, 2 co-halves)
  W2[co, pix, hl]  = W.transpose(3,0,1,2).reshape(9216,160) re-indexed
  s[b,hl]  = sum_{co,pix} c[n] * xr * W2          (72 PE matmuls, K=128)
  v        = squash_dim1(s)
  P[co,pix,b] = sum_hl W2 * v[b,hl]               (PE)
  delta[n] = 1/(B*160) * sum_{s,b} xr * P         (DVE + mask matmul)
Convs are PE matmuls: conv1 via host-im2col patches (K=81), primary-caps
conv via 81 shifted-window matmuls accumulated in PSUM (K=256 as 2x128).
"""

import numpy as np

import concourse.bass as bass
import concourse.mybir as mybir
import concourse.tile as tile
from concourse.ap import AP
from concourse.bass_utils import run_bass_kernel_spmd

F32 = mybir.dt.float32
F16 = mybir.dt.float16
AL = mybir.AluOpType
AF = mybir.ActivationFunctionType
AX = mybir.AxisListType

NCORES = 8
B = 512
BC = B // NCORES           # 64 images per core
MAX_WAITS = 1              # walrus on this path allows 1 sync wait per inst
HL = 160                   # 10 classes x 16 pose
GROUPS = [(0, 14), (14, 14), (28, 14), (42, 14), (56, 8)]  # conv2 image groups
ROUTE_SCALE = 1.0 / (B * HL)
CHUNK = 8                  # conv1 images per im2col DMA chunk


def _r(t, dims):
    """Raw AP on tile/ap t with explicit [step, count] dims (elements)."""
    return AP(t.tensor, t.offset, dims)


def split_waits(nc, max_waits=MAX_WAITS):
    """This walrus build rejects >max_waits sync waits per instruction; move
    excess waits onto same-engine NoOps inserted immediately before."""
    for f in nc.m.functions:
        for blk in f.blocks:
            out = []
            for ins in blk.instructions:
                si = ins.sync_info
                if si is not None and si.on_wait and len(si.on_wait) > max_waits:
                    waits = list(si.on_wait)
                    k = 0
                    while len(waits) > max_waits:
                        chunk, waits = waits[:max_waits], waits[max_waits:]
                        nop = mybir.InstNoOp(name=f"{ins.name}-ws{k}", ins=[], outs=[])
                        nop.engine = ins.engine
                        nop.sync_info = mybir.SyncInfo(on_wait=chunk, on_update=[])
                        out.append(nop)
                        k += 1
                    ins.sync_info = mybir.SyncInfo(
                        on_wait=waits, on_update=list(si.on_update or []))
                out.append(ins)
            blk.instructions = out


def build_nc():
    nc = bass.Bass(num_devices=NCORES)

    xpatch = nc.dram_tensor("xpatch", [81, BC * 560], F16, kind="ExternalInput")
    w1t = nc.dram_tensor("w1t", [81, 256], F16, kind="ExternalInput")
    b1 = nc.dram_tensor("b1", [256], F32, kind="ExternalInput")
    # conv2 weights, retiled: [co_blk, ci_blk, 128ci, 81kk * 128co]
    pcw4 = nc.dram_tensor("pcw4", [2, 2, 128, 81 * 128], F16,
                          kind="ExternalInput")
    pcb = nc.dram_tensor("pcb", [256], F32, kind="ExternalInput")
    # routing weights in (co_blk, pix) tiling
    w2p = nc.dram_tensor("w2p", [2, 128, 36 * HL], F16, kind="ExternalInput")
    w2pt_a = nc.dram_tensor("w2pt_a", [2, 128, 36 * 128], F16,
                            kind="ExternalInput")
    w2pt_b = nc.dram_tensor("w2pt_b", [2, 32, 36 * 128], F16,
                            kind="ExternalInput")
    eye64 = nc.dram_tensor("eye64", [BC, BC], F16, kind="ExternalInput")
    maskT = nc.dram_tensor("maskT", [128, 32], F32, kind="ExternalInput")
    mask32 = nc.dram_tensor("mask32", [32, 128], F32, kind="ExternalInput")
    vout = nc.dram_tensor("vout", [BC, HL], F32, kind="ExternalOutput")

    with tile.TileContext(nc) as tc:
        with (
            tc.tile_pool(name="pers", bufs=1) as pers,
            tc.tile_pool(name="dram", bufs=1, space="DRAM") as dpool,
            tc.tile_pool(name="sps", bufs=1, space="PSUM") as sps,
        ):
            w1t_sb = pers.tile([81, 256], F16)
            nc.sync.dma_start(w1t_sb[:], w1t[:])
            b1_sb = pers.tile([128, 2], F32)
            nc.sync.dma_start(b1_sb[:], _r(b1[:], [[1, 128], [128, 2]]))
            pcb_sb = pers.tile([128, 2], F32)
            nc.sync.dma_start(pcb_sb[:], _r(pcb[:], [[1, 128], [128, 2]]))
            ones32 = pers.tile([32, 1], F32)
            nc.gpsimd.memset(ones32[:], 1.0)
            ones1 = pers.tile([1, 32], F32)
            nc.gpsimd.memset(ones1[:], 1.0)
            b32 = pers.tile([32, 36], F32)
            eye_sb = pers.tile([BC, BC], F16)
            maskT_sb = pers.tile([128, 32], F32)
            mask32_sb = pers.tile([32, 128], F32)
            # routing s-weights [co_blk][128, 36*160]; prefetched during conv1
            w2p_sb = [pers.tile([128, 36 * HL], F16, tag=f"w2p{cb}",
                                name=f"w2p{cb}") for cb in range(2)]
            # xr (primary caps output), written by conv2 epilogue
            xr_sb = [pers.tile([128, 36 * BC], F16, tag=f"xr{cb}",
                               name=f"xr{cb}") for cb in range(2)]

            # ---------------- conv phase ----------------
            with (
                tc.tile_pool(name="h1p", bufs=1) as h1p,
                tc.tile_pool(name="w2cp", bufs=2) as w2cp,
            ):
                h1s = [h1p.tile([128, BC * 400], F16, tag=f"h1_{ci}",
                                name=f"h1_{ci}")
                       for ci in range(2)]

                def load_w2c(co, ci, eng):
                    t = w2cp.tile([128, 81 * 128], F16, tag="w2c")
                    eng.dma_start(
                        t[:],
                        AP(pcw4[:].tensor, (co * 2 + ci) * 128 * 81 * 128,
                           [[81 * 128, 128], [1, 81 * 128]]),
                    )
                    return t

                with (
                    tc.tile_pool(name="pwp", bufs=3) as pwp,
                    tc.tile_pool(name="ps1p", bufs=2, space="PSUM") as ps1p,
                    tc.tile_pool(name="ps2p", bufs=1, space="PSUM") as ps2p,
                ):
                    NCH = BC // CHUNK
                    pas = {}
                    n_warm = 0

                    def load_chunk(k, eng):
                        pa = pwp.tile([81, CHUNK * 560], F16, tag="pa")
                        eng.dma_start(
                            pa[:],
                            AP(xpatch[:].tensor, k * CHUNK * 560,
                               [[BC * 560, 81], [1, CHUNK * 560]]),
                        )
                        pas[k] = pa

                    def conv1_pair(k, lp):
                        """conv1 for an image pair: 2 matmuls into one
                        bank-padded PSUM tile per ci, then ONE batched relu
                        per (pair, ci), Scalar / DVE split."""
                        pa = pas[k]
                        pstep = pa.ap[0][0]
                        gi = k * CHUNK + lp * 2
                        for ci in range(2):
                            ps = ps1p.tile([128, 1024], F32, tag="ps1")
                            pp = ps.ap[0][0]
                            for u in range(2):
                                rhs = AP(pa.tensor,
                                         pa.offset + (lp * 2 + u) * 560,
                                         [[pstep, 81], [28, 20], [1, 20]])
                                nc.tensor.matmul(
                                    AP(ps.tensor, ps.offset + u * 512,
                                       [[pp, 128], [1, 400]]),
                                    w1t_sb[:, ci * 128:(ci + 1) * 128],
                                    rhs,
                                    start=True, stop=True,
                                )
                            dst = AP(h1s[ci].tensor,
                                     h1s[ci].offset + gi * 400,
                                     [[h1s[ci].ap[0][0], 128],
                                      [400, 2], [1, 400]])
                            src = _r(ps, [[pp, 128], [512, 2], [1, 400]])
                            if ci == 0:
                                nc.scalar.activation(
                                    dst, src, AF.Relu,
                                    bias=b1_sb[:, ci:ci + 1],
                                )
                            else:
                                nc.vector.tensor_scalar(
                                    dst, src, b1_sb[:, ci:ci + 1], 0.0,
                                    AL.add, AL.max,
                                )

                    def conv1_chunk(k):
                        for lp in range(CHUNK // 2):
                            conv1_pair(k, lp)
                        pas.pop(k)

                    def conv2_group(co_blk, g, g0, nb, w2cs, interleave=()):
                        """One conv2 output group; optionally interleaves
                        pending conv1 pair thunks into the matmul stream so
                        their epilogues drain during conv2's long PSUM
                        accumulation (no conv1 stalls)."""
                        interleave = list(interleave)
                        stride = max(1, (162 // (len(interleave) + 1))
                                     if interleave else 162)
                        ps2 = ps2p.tile([128, 504], F32, tag="ps2", bufs=2)
                        pstep = ps2.ap[0][0]
                        out4 = _r(ps2, [[pstep, 128], [36, nb], [6, 6], [1, 6]])
                        n_mm = 0
                        for ci in range(2):
                            h1 = h1s[ci]
                            hp = h1.ap[0][0]
                            for kk in range(81):
                                ky, kx = divmod(kk, 9)
                                rhs = AP(h1.tensor,
                                         h1.offset + g0 * 400 + ky * 20 + kx,
                                         [[hp, 128], [400, nb], [40, 6], [2, 6]])
                                nc.tensor.matmul(
                                    out4,
                                    w2cs[ci][:, kk * 128:(kk + 1) * 128],
                                    rhs,
                                    start=(ci == 0 and kk == 0),
                                    stop=(ci == 1 and kk == 80),
                                )
                                n_mm += 1
                                if interleave and n_mm % stride == 0:
                                    interleave.pop(0)()
                        for t in interleave:
                            t()
                        # bias+relu, write pix-major (col = pix*BC + b)
                        xr = xr_sb[co_blk]
                        nc.scalar.activation(
                            AP(xr.tensor, xr.offset + g0,
                               [[xr.ap[0][0], 128], [1, nb], [BC, 36]]),
                            _r(ps2, [[pstep, 128], [36, nb], [1, 36]]),
                            AF.Relu,
                            bias=pcb_sb[:, co_blk:co_blk + 1],
                        )

                    def warmup_cc(dep_ap):
                        """Collective keyed on conv progress: absorbs
                        inter-core skew off the critical path so the real
                        AllReduces are fast."""
                        nonlocal n_warm
                        cinw = dpool.tile([128, 1], F32, name=f"cinw{n_warm}")
                        coutw = dpool.tile([128, 1], F32, name=f"coutw{n_warm}",
                                           addr_space="Shared")
                        n_warm += 1
                        nc.gpsimd.dma_start(cinw[:], dep_ap)
                        nc.gpsimd.collective_compute(
                            "AllReduce", AL.add,
                            replica_groups=[list(range(NCORES))],
                            ins=[cinw.opt()], outs=[coutw.opt()],
                        )

                    # group-pipelined conv: conv1 chunks feed conv2 co0
                    # groups, with later chunks' image-pairs interleaved INTO
                    # the conv2 matmul stream. ALL bulk DMAs ride the sync
                    # queue so the scalar engine stays free for epilogues.
                    load_chunk(0, nc.sync)
                    load_chunk(1, nc.sync)
                    w2c0 = [load_w2c(0, 0, nc.sync)]
                    load_chunk(2, nc.sync)
                    conv1_chunk(0)
                    load_chunk(3, nc.sync)
                    conv1_chunk(1)
                    w2c0.append(load_w2c(0, 1, nc.sync))
                    # small routing constants, nothing urgent
                    nc.sync.dma_start(eye_sb[:], eye64[:])
                    nc.sync.dma_start(maskT_sb[:], maskT[:])
                    nc.sync.dma_start(mask32_sb[:], mask32[:])
                    for cb in range(2):
                        nc.sync.dma_start(
                            w2p_sb[cb][:],
                            AP(w2p[:].tensor, cb * 128 * 36 * HL,
                               [[36 * HL, 128], [1, 36 * HL]]),
                        )
                    load_chunk(4, nc.sync)

                    def pairs(k, then_load=None):
                        out = []
                        for lp in range(CHUNK // 2):
                            def thunk(k=k, lp=lp, last=(lp == CHUNK // 2 - 1)):
                                conv1_pair(k, lp)
                                if last:
                                    pas.pop(k)
                                    if then_load is not None:
                                        load_chunk(then_load, nc.sync)
                            out.append(thunk)
                        return out

                    ilv = {0: pairs(2, 5) + pairs(3, 6),
                           1: pairs(4, 7) + pairs(5),
                           2: pairs(6), 3: pairs(7), 4: []}
                    for g, (g0, nb) in enumerate(GROUPS):
                        conv2_group(0, g, g0, nb, w2c0, ilv[g])
                        if g == 0:
                            warmup_cc(xr_sb[0][:, 0:1])
                    # round-1 s-matmul, co0 half: c is uniform in round 1, so
                    # these fold into the conv stream as soon as xr0 is done
                    s_ps1 = sps.tile([BC, HL], F32, tag="s_ps")
                    for pix in range(36):
                        nc.tensor.matmul(
                            s_ps1[:],
                            xr_sb[0][:, pix * BC:(pix + 1) * BC],
                            w2p_sb[0][:, pix * HL:(pix + 1) * HL],
                            start=(pix == 0), stop=False,
                        )
                    w2c1 = [load_w2c(1, 0, nc.sync), load_w2c(1, 1, nc.scalar)]
                    for g, (g0, nb) in enumerate(GROUPS):
                        conv2_group(1, g, g0, nb, w2c1)
                        if g == 0:
                            warmup_cc(xr_sb[1][:, 0:1])

            # ---------------- routing phase ----------------
            with (
                tc.tile_pool(name="rsb", bufs=1) as rsb,
                tc.tile_pool(name="rnd", bufs=2) as rnd,
                tc.tile_pool(name="gps", bufs=4, space="PSUM") as gps,
                tc.tile_pool(name="zps", bufs=1, space="PSUM") as zps,
            ):
                # W2^T for the P matmuls: loaded at routing start (h1 freed)
                w2pt_a_sb = [rsb.tile([128, 36 * 128], F16, tag=f"w2pta{cb}",
                                      name=f"w2pta{cb}") for cb in range(2)]
                for cb in range(2):
                    (nc.sync if cb == 0 else nc.scalar).dma_start(
                        w2pt_a_sb[cb][:],
                        AP(w2pt_a[:].tensor, cb * 128 * 36 * 128,
                           [[36 * 128, 128], [1, 36 * 128]]),
                    )
                w2pt_b_sb = [rsb.tile([32, 36 * 128], F16, tag=f"w2ptb{cb}",
                                      name=f"w2ptb{cb}") for cb in range(2)]
                for cb in range(2):
                    (nc.sync if cb == 0 else nc.scalar).dma_start(
                        w2pt_b_sb[cb][:],
                        AP(w2pt_b[:].tensor, cb * 32 * 36 * 128,
                           [[36 * 128, 32], [1, 36 * 128]]),
                    )
                prod = rsb.tile([128, 2 * 36 * BC], F16)

                def s_matmul():
                    s_ps = sps.tile([BC, HL], F32, tag="s_ps")
                    first, last = (0, 0), (1, 35)
                    for cb in range(2):
                        for pix in range(36):
                            nc.tensor.matmul(
                                s_ps[:],
                                xr_sb[cb][:, pix * BC:(pix + 1) * BC],
                                w2p_sb[cb][:, pix * HL:(pix + 1) * HL],
                                start=((cb, pix) == first),
                                stop=((cb, pix) == last),
                            )
                    return s_ps

                def squash(s_sb, out_dtype):
                    sq = rnd.tile([BC, HL], F32, tag="sq")
                    nc.scalar.square(sq[:], s_sb[:])
                    n2 = rnd.tile([BC, 16], F32, tag="n2")
                    nc.vector.tensor_reduce(
                        n2[:].rearrange("a b -> a b ()"),
                        _r(sq, [[sq.ap[0][0], BC], [1, 16], [16, 10]]),
                        AX.X, AL.add,
                    )
                    rt = rnd.tile([BC, 16], F32, tag="rt")
                    nc.scalar.sqrt(rt[:], n2[:])
                    n2p1 = rnd.tile([BC, 16], F32, tag="n2p1")
                    nc.vector.tensor_scalar_add(n2p1[:], n2[:], 1.0)
                    rcp = rnd.tile([BC, 16], F32, tag="rcp")
                    nc.vector.reciprocal(rcp[:], n2p1[:])
                    f = rnd.tile([BC, 16], F32, tag="f")
                    nc.vector.tensor_tensor(f[:], rt[:], rcp[:], AL.mult)
                    v_sb = rnd.tile([BC, HL], out_dtype, tag="v_sb")
                    nc.vector.tensor_tensor(
                        _r(v_sb, [[v_sb.ap[0][0], BC], [16, 10], [1, 16]]),
                        _r(s_sb, [[s_sb.ap[0][0], BC], [16, 10], [1, 16]]),
                        _r(f, [[f.ap[0][0], BC], [0, 10], [1, 16]]),
                        AL.mult,
                    )
                    return v_sb

                def p_delta_update(v16, rnd_idx, rce32):
                    """delta via P[co,pix,b] = sum_hl W2*v (PE), then
                    D[co,pix] = sum_b xr*P (DVE), then delta32[c32,pix] =
                    mask-matmul partition regroup. If xr is c-scaled, divide
                    by ce32 (rce32 ap) to undo."""
                    vt_ps = gps.tile([128, 2 * BC], F16, tag="vt_ps", bufs=1)
                    nc.tensor.transpose(vt_ps[:, 0:BC], v16[:, 0:128], eye_sb[:])
                    nc.tensor.transpose(
                        AP(vt_ps.tensor, vt_ps.offset + BC,
                           [[vt_ps.ap[0][0], 32], [1, BC]]),
                        v16[:, 128:160], eye_sb[:])
                    vt_a = rnd.tile([128, BC], F16, tag="vt_a")
                    nc.scalar.copy(vt_a[:], vt_ps[:, 0:BC])
                    vt_b = rnd.tile([32, BC], F16, tag="vt_b")
                    nc.scalar.copy(
                        vt_b[:],
                        AP(vt_ps.tensor, vt_ps.offset + BC,
                           [[vt_ps.ap[0][0], 32], [1, BC]]))
                    # P in 4-pix batches; DVE multiplies straight out of PSUM
                    TB = 4
                    for cb in range(2):
                        for pb in range(36 // TB):
                            p_ps = gps.tile([128, TB * BC], F32, tag="p_ps",
                                            bufs=2)
                            for j in range(TB):
                                pix = pb * TB + j
                                nc.tensor.matmul(
                                    p_ps[:, j * BC:(j + 1) * BC],
                                    w2pt_a_sb[cb][:, pix * 128:(pix + 1) * 128],
                                    vt_a[:],
                                    start=True, stop=False,
                                )
                                nc.tensor.matmul(
                                    p_ps[:, j * BC:(j + 1) * BC],
                                    w2pt_b_sb[cb][:, pix * 128:(pix + 1) * 128],
                                    vt_b[:],
                                    start=False, stop=True,
                                )
                            xh = xr_sb[cb]
                            nc.vector.tensor_tensor(
                                prod[:, (cb * 36 + pb * TB) * BC:
                                     (cb * 36 + pb * TB + TB) * BC],
                                AP(xh.tensor, xh.offset + pb * TB * BC,
                                   [[xh.ap[0][0], 128], [1, TB * BC]]),
                                p_ps[:],
                                AL.mult,
                            )
                    ds_ps = gps.tile([32, 36], F32, tag="ds_ps", bufs=1)
                    for cb in range(2):
                        D = rnd.tile([128, 36], F32, tag=f"D{cb}")
                        nc.vector.tensor_reduce(
                            D[:].rearrange("a b -> a b ()"),
                            AP(prod.tensor, prod.offset + cb * 36 * BC,
                               [[prod.ap[0][0], 128], [BC, 36], [1, BC]]),
                            AX.X, AL.add,
                        )
                        # regroup: delta32[c32,pix] = sum_{p: p%32==c32} D[p,pix]
                        nc.tensor.matmul(
                            ds_ps[:], maskT_sb[:], D[:],
                            start=(cb == 0), stop=(cb == 1),
                        )
                    delta32 = rnd.tile([32, 36], F32, tag="delta32")
                    if rce32 is not None:
                        nc.vector.tensor_tensor(
                            delta32[:], ds_ps[:], rce32[:], AL.mult)
                    else:
                        nc.scalar.copy(delta32[:], ds_ps[:])
                    cin = dpool.tile([32, 36], F32, name=f"cin{rnd_idx}")
                    cout = dpool.tile([32, 36], F32, name=f"cout{rnd_idx}",
                                      addr_space="Shared")
                    nc.gpsimd.dma_start(cin[:], delta32[:])
                    nc.gpsimd.collective_compute(
                        "AllReduce", AL.add,
                        replica_groups=[list(range(NCORES))],
                        ins=[cin.opt()], outs=[cout.opt()],
                    )
                    dsum = rnd.tile([32, 36], F32, tag="dsum")
                    nc.gpsimd.dma_start(dsum[:], cout[:])
                    if rnd_idx == 0:
                        nc.scalar.mul(b32[:], dsum[:], ROUTE_SCALE)
                    else:
                        sc = rnd.tile([32, 36], F32, tag="sc")
                        nc.scalar.mul(sc[:], dsum[:], ROUTE_SCALE)
                        nc.vector.tensor_tensor(b32[:], b32[:], sc[:], AL.add)

                def softmax_ce():
                    """ce32[c32,pix] = softmax(b32)[n=c32*36+pix], F32."""
                    e32 = rnd.tile([32, 36], F32, tag="e32")
                    nc.scalar.activation(e32[:], b32[:], AF.Exp)
                    rs = rnd.tile([32, 1], F32, tag="rs")
                    nc.vector.tensor_reduce(
                        rs[:].rearrange("a b -> a b ()"), e32[:], AX.X, AL.add)
                    z_ps = zps.tile([1, 1], F32, tag="z_ps")
                    nc.tensor.matmul(z_ps[:], ones32[:], rs[:], start=True, stop=True)
                    z_sb = rnd.tile([1, 1], F32, tag="z_sb")
                    nc.scalar.copy(z_sb[:], z_ps[:])
                    zb_ps = zps.tile([32, 1], F32, tag="zb_ps")
                    nc.tensor.matmul(zb_ps[:], ones1[:], z_sb[:], start=True, stop=True)
                    rz = rnd.tile([32, 1], F32, tag="rz")
                    nc.vector.reciprocal(rz[:], zb_ps[:])
                    ce32 = rnd.tile([32, 36], F32, tag="ce32")
                    nc.vector.tensor_scalar_mul(ce32[:], e32[:], rz[:])
                    return ce32

                def scale_xr(m32f32):
                    """xr[co, pix, b] *= m32[co%32, pix] in place."""
                    cm_ps = zps.tile([128, 36], F32, tag="cm_ps")
                    nc.tensor.matmul(cm_ps[:], mask32_sb[:], m32f32[:],
                                     start=True, stop=True)
                    for cb in range(2):
                        xh = xr_sb[cb]
                        nc.vector.tensor_tensor(
                            _r(xh, [[xh.ap[0][0], 128], [BC, 36], [1, BC]]),
                            _r(xh, [[xh.ap[0][0], 128], [BC, 36], [1, BC]]),
                            _r(cm_ps, [[cm_ps.ap[0][0], 128], [1, 36], [0, BC]]),
                            AL.mult,
                        )

                # ---- round 1 (c uniform; xr unscaled) ----
                # co0 half was accumulated into s_ps1 during the conv phase
                for pix in range(36):
                    nc.tensor.matmul(
                        s_ps1[:],
                        xr_sb[1][:, pix * BC:(pix + 1) * BC],
                        w2p_sb[1][:, pix * HL:(pix + 1) * HL],
                        start=False, stop=(pix == 35),
                    )
                s_sb = rnd.tile([BC, HL], F32, tag="s_sb")
                nc.scalar.mul(s_sb[:], s_ps1[:], 1.0 / 1152.0)
                v16 = squash(s_sb, F16)
                p_delta_update(v16, 0, None)
                # ---- round 2 ----
                ce2 = softmax_ce()
                scale_xr(ce2)
                s_ps = s_matmul()
                # off the critical path: runs on DVE while the PE streams s
                rce32 = rnd.tile([32, 36], F32, tag="rce32")
                nc.vector.reciprocal(rce32[:], ce2[:])
                s_sb = rnd.tile([BC, HL], F32, tag="s_sb")
                nc.scalar.copy(s_sb[:], s_ps[:])
                v16 = squash(s_sb, F16)
                p_delta_update(v16, 1, rce32)
                # ---- round 3 (b update dead) ----
                ce3 = softmax_ce()
                ratio32 = rnd.tile([32, 36], F32, tag="ratio32")
                nc.vector.tensor_tensor(ratio32[:], ce3[:], rce32[:], AL.mult)
                scale_xr(ratio32)
                s_ps = s_matmul()
                s_sb = rnd.tile([BC, HL], F32, tag="s_sb")
                nc.scalar.copy(s_sb[:], s_ps[:])
                v_sb = squash(s_sb, F32)
                nc.sync.dma_start(vout[:], v_sb[:])

    return nc


_NC_CACHE = None


def _get_nc():
    global _NC_CACHE
    if _NC_CACHE is None:
        nc = build_nc()
        split_waits(nc)
        _NC_CACHE = nc
    return _NC_CACHE


def prepare_inputs(x, conv1_w, conv1_b, pc_w, pc_b, W):
    x = np.asarray(x, np.float32)
    xf = np.zeros((B, 800), np.float16)
    xf[:, :784] = x.reshape(B, 784).astype(np.float16)
    # host-side im2col ("wide patch"): xp[i, (ky,kx), j] = xf[i, 28*ky+kx+j]
    xp = np.lib.stride_tricks.as_strided(
        xf, shape=(B, 9, 9, 560), strides=(1600, 56, 2, 2)).reshape(B, 81, 560)
    w1t = np.ascontiguousarray(
        np.asarray(conv1_w, np.float32).reshape(256, 81).T).astype(np.float16)
    b1 = np.ascontiguousarray(np.asarray(conv1_b, np.float32))
    # pcw4[co_blk, ci_blk, ci128, kk*128co] = pc_w[co, ci, ky, kx]
    pcw = np.asarray(pc_w, np.float32).reshape(256, 256, 81)  # [co, ci, kk]
    pcw4 = np.ascontiguousarray(
        pcw.reshape(2, 128, 2, 128, 81).transpose(0, 2, 3, 4, 1)
    ).astype(np.float16)  # [co_blk, ci_blk, ci128, kk, co128]
    pcb = np.ascontiguousarray(np.asarray(pc_b, np.float32).reshape(256))
    # W2cp[co, pix, hl] = W2n[co*36+pix, hl]
    w2n = np.asarray(W, np.float32).transpose(3, 0, 1, 2).reshape(9216, HL)
    w2cp = w2n.reshape(256, 36, HL)
    w2p = np.ascontiguousarray(
        w2cp.reshape(2, 128, 36 * HL)).astype(np.float16)
    w2t = w2cp.transpose(2, 1, 0)                 # [hl, pix, co]
    w2pt_a = np.ascontiguousarray(
        w2t[:128].reshape(128, 36, 2, 128).transpose(2, 0, 1, 3)
        .reshape(2, 128, 36 * 128)).astype(np.float16)
    w2pt_b = np.ascontiguousarray(
        w2t[128:].reshape(32, 36, 2, 128).transpose(2, 0, 1, 3)
        .reshape(2, 32, 36 * 128)).astype(np.float16)
    eye64 = np.eye(BC, dtype=np.float16)
    maskT = np.zeros((128, 32), np.float32)
    maskT[np.arange(128), np.arange(128) % 32] = 1.0
    mask32 = np.ascontiguousarray(maskT.T)
    in_maps = []
    for c in range(NCORES):
        in_maps.append({
            "xpatch": np.ascontiguousarray(
                xp[c * BC:(c + 1) * BC].transpose(1, 0, 2).reshape(81, BC * 560)),
            "w1t": w1t, "b1": b1, "pcw4": pcw4, "pcb": pcb, "w2p": w2p,
            "w2pt_a": w2pt_a, "w2pt_b": w2pt_b, "eye64": eye64,
            "maskT": maskT, "mask32": mask32,
        })
    return in_maps


def kernel(x, conv1_w, conv1_b, pc_w, pc_b, W, _trace=False, _trace_kwargs=None):
    nc = _get_nc()
    in_maps = prepare_inputs(x, conv1_w, conv1_b, pc_w, pc_b, W)
    res = run_bass_kernel_spmd(
        nc, in_maps, list(range(NCORES)),
        trace=_trace, **(_trace_kwargs or {}),
    )
    v = np.concatenate([np.asarray(res.results[c]["vout"]) for c in range(NCORES)], 0)
    out = v.reshape(B, 1, 1, 10, 16).astype(np.float32)
    if _trace:
        return out, res
    return out


# revision 65
# speedup vs baseline: 1.0421x; 1.0266x over previous
"""CapsNet forward kernel for Trainium2, 8-core data-parallel.

Strategy (per spec sharding_hint): batch (512) split across 8 cores (64 each);
all params replicated. Routing logits b are a batch-mean -> AllReduce of
per-core partial deltas (1152 floats) per routing round (rounds 1,2 only;
round 3's b update is dead in the reference).

Optimizations (1156us baseline -> ~695us):
- all big matmuls fp16 (1 cycle/row on the PE vs 2 for fp32-HIGH mode)
- host-side im2col for conv1; host-retiled weights so every weight DMA is
  contiguous (128 descriptors instead of ~10k strided ones)
- conv1 fully interleaved into the conv2 matmul stream (image-pair matmuls
  + batched Scalar/DVE relu epilogues between conv2 groups)
- routing contraction tiled by (co_blk, pix) so it consumes the primary-caps
  output directly from SBUF (no DRAM round-trip / scatter DMAs); routing
  logits kept as [32, 36] (n = c32*36 + pix) with tiny mask-matmuls for the
  cross-partition regroups
- round-1 s-matmul co0 half folded into the conv phase (c is uniform)
- one AllReduce per round (rounds 1-2), warm-up collectives during conv to
  absorb cold-start + inter-core skew
- P-phase products consumed by DVE straight out of PSUM (no staging copies)

Math (keeps exact semantics, never materializes u):
  n = c32*36 + pix, co = s*32 + c32, r = s*1152 + n
  xr[co, pix, b]   primary-caps relu output (SBUF, 2 co-halves)
  W2[co, pix, hl]  = W.transpose(3,0,1,2).reshape(9216,160) re-indexed
  s[b,hl]  = sum_{co,pix} c[n] * xr * W2          (72 PE matmuls, K=128)
  v        = squash_dim1(s)
  P[co,pix,b] = sum_hl W2 * v[b,hl]               (PE)
  delta[n] = 1/(B*160) * sum_{s,b} xr * P         (DVE + mask matmul)
Convs are PE matmuls: conv1 via host-im2col patches (K=81), primary-caps
conv via 81 shifted-window matmuls accumulated in PSUM (K=256 as 2x128).
"""

import numpy as np

import concourse.bass as bass
import concourse.mybir as mybir
import concourse.tile as tile
from concourse.ap import AP
from concourse.bass_utils import run_bass_kernel_spmd

F32 = mybir.dt.float32
F16 = mybir.dt.float16
AL = mybir.AluOpType
AF = mybir.ActivationFunctionType
AX = mybir.AxisListType

NCORES = 8
B = 512
BC = B // NCORES           # 64 images per core
MAX_WAITS = 1              # walrus on this path allows 1 sync wait per inst
HL = 160                   # 10 classes x 16 pose
GROUPS = [(0, 14), (14, 14), (28, 14), (42, 14), (56, 8)]  # conv2 image groups
ROUTE_SCALE = 1.0 / (B * HL)
CHUNK = 8                  # conv1 images per im2col DMA chunk


def _r(t, dims):
    """Raw AP on tile/ap t with explicit [step, count] dims (elements)."""
    return AP(t.tensor, t.offset, dims)


def split_waits(nc, max_waits=MAX_WAITS):
    """This walrus build rejects >max_waits sync waits per instruction; move
    excess waits onto same-engine NoOps inserted immediately before."""
    for f in nc.m.functions:
        for blk in f.blocks:
            out = []
            for ins in blk.instructions:
                si = ins.sync_info
                if si is not None and si.on_wait and len(si.on_wait) > max_waits:
                    waits = list(si.on_wait)
                    k = 0
                    while len(waits) > max_waits:
                        chunk, waits = waits[:max_waits], waits[max_waits:]
                        nop = mybir.InstNoOp(name=f"{ins.name}-ws{k}", ins=[], outs=[])
                        nop.engine = ins.engine
                        nop.sync_info = mybir.SyncInfo(on_wait=chunk, on_update=[])
                        out.append(nop)
                        k += 1
                    ins.sync_info = mybir.SyncInfo(
                        on_wait=waits, on_update=list(si.on_update or []))
                out.append(ins)
            blk.instructions = out


def build_nc():
    nc = bass.Bass(num_devices=NCORES)

    xpatch = nc.dram_tensor("xpatch", [81, BC * 560], F16, kind="ExternalInput")
    w1t = nc.dram_tensor("w1t", [81, 256], F16, kind="ExternalInput")
    b1 = nc.dram_tensor("b1", [256], F32, kind="ExternalInput")
    # conv2 weights, retiled: [co_blk, ci_blk, 128ci, 81kk * 128co]
    pcw4 = nc.dram_tensor("pcw4", [2, 2, 128, 81 * 128], F16,
                          kind="ExternalInput")
    pcb = nc.dram_tensor("pcb", [256], F32, kind="ExternalInput")
    # routing weights in (co_blk, pix) tiling
    w2p = nc.dram_tensor("w2p", [2, 128, 36 * HL], F16, kind="ExternalInput")
    w2pt_a = nc.dram_tensor("w2pt_a", [2, 128, 36 * 128], F16,
                            kind="ExternalInput")
    # hl-tail (hl 128..159) weights packed 2-pix-per-K-tile for the
    # block-diagonal P_b matmul: rows (q, hl'), cols (pp2, co)
    w2ptb2 = nc.dram_tensor("w2ptb2", [2, 64, 18 * 128], F16,
                            kind="ExternalInput")
    eye64 = nc.dram_tensor("eye64", [BC, BC], F16, kind="ExternalInput")
    maskT = nc.dram_tensor("maskT", [128, 32], F32, kind="ExternalInput")
    mask32 = nc.dram_tensor("mask32", [32, 128], F32, kind="ExternalInput")
    vout = nc.dram_tensor("vout", [BC, HL], F32, kind="ExternalOutput")

    with tile.TileContext(nc) as tc:
        with (
            tc.tile_pool(name="pers", bufs=1) as pers,
            tc.tile_pool(name="dram", bufs=1, space="DRAM") as dpool,
            tc.tile_pool(name="sps", bufs=1, space="PSUM") as sps,
        ):
            w1t_sb = pers.tile([81, 256], F16)
            nc.sync.dma_start(w1t_sb[:], w1t[:])
            b1_sb = pers.tile([128, 2], F32)
            nc.sync.dma_start(b1_sb[:], _r(b1[:], [[1, 128], [128, 2]]))
            pcb_sb = pers.tile([128, 2], F32)
            nc.sync.dma_start(pcb_sb[:], _r(pcb[:], [[1, 128], [128, 2]]))
            ones32 = pers.tile([32, 1], F32)
            nc.gpsimd.memset(ones32[:], 1.0)
            ones1 = pers.tile([1, 32], F32)
            nc.gpsimd.memset(ones1[:], 1.0)
            b32 = pers.tile([32, 36], F32)
            eye_sb = pers.tile([BC, BC], F16)
            maskT_sb = pers.tile([128, 32], F32)
            mask32_sb = pers.tile([32, 128], F32)
            # routing s-weights [co_blk][128, 36*160]; prefetched during conv1
            w2p_sb = [pers.tile([128, 36 * HL], F16, tag=f"w2p{cb}",
                                name=f"w2p{cb}") for cb in range(2)]
            # xr (primary caps output), written by conv2 epilogue
            xr_sb = [pers.tile([128, 36 * BC], F16, tag=f"xr{cb}",
                               name=f"xr{cb}") for cb in range(2)]
            # block-diagonal diag2(v[:,128:160]^T): off-diagonal stays zero
            # forever; each round only rewrites the diagonal blocks
            vb2 = pers.tile([64, 2 * BC], F16)
            nc.gpsimd.memset(vb2[:], 0.0)

            # ---------------- conv phase ----------------
            with (
                tc.tile_pool(name="h1p", bufs=1) as h1p,
                tc.tile_pool(name="w2cp", bufs=2) as w2cp,
            ):
                h1s = [h1p.tile([128, BC * 400], F16, tag=f"h1_{ci}",
                                name=f"h1_{ci}")
                       for ci in range(2)]

                def load_w2c(co, ci, eng):
                    t = w2cp.tile([128, 81 * 128], F16, tag="w2c")
                    eng.dma_start(
                        t[:],
                        AP(pcw4[:].tensor, (co * 2 + ci) * 128 * 81 * 128,
                           [[81 * 128, 128], [1, 81 * 128]]),
                    )
                    return t

                with (
                    tc.tile_pool(name="pwp", bufs=3) as pwp,
                    tc.tile_pool(name="ps1p", bufs=2, space="PSUM") as ps1p,
                    tc.tile_pool(name="ps2p", bufs=1, space="PSUM") as ps2p,
                ):
                    NCH = BC // CHUNK
                    pas = {}
                    n_warm = 0

                    def load_chunk(k, eng):
                        pa = pwp.tile([81, CHUNK * 560], F16, tag="pa")
                        eng.dma_start(
                            pa[:],
                            AP(xpatch[:].tensor, k * CHUNK * 560,
                               [[BC * 560, 81], [1, CHUNK * 560]]),
                        )
                        pas[k] = pa

                    def conv1_pair(k, lp):
                        """conv1 for an image pair: 2 matmuls into one
                        bank-padded PSUM tile per ci, then ONE batched relu
                        per (pair, ci), Scalar / DVE split."""
                        pa = pas[k]
                        pstep = pa.ap[0][0]
                        gi = k * CHUNK + lp * 2
                        for ci in range(2):
                            ps = ps1p.tile([128, 1024], F32, tag="ps1")
                            pp = ps.ap[0][0]
                            for u in range(2):
                                rhs = AP(pa.tensor,
                                         pa.offset + (lp * 2 + u) * 560,
                                         [[pstep, 81], [28, 20], [1, 20]])
                                nc.tensor.matmul(
                                    AP(ps.tensor, ps.offset + u * 512,
                                       [[pp, 128], [1, 400]]),
                                    w1t_sb[:, ci * 128:(ci + 1) * 128],
                                    rhs,
                                    start=True, stop=True,
                                )
                            dst = AP(h1s[ci].tensor,
                                     h1s[ci].offset + gi * 400,
                                     [[h1s[ci].ap[0][0], 128],
                                      [400, 2], [1, 400]])
                            src = _r(ps, [[pp, 128], [512, 2], [1, 400]])
                            if ci == 0:
                                nc.scalar.activation(
                                    dst, src, AF.Relu,
                                    bias=b1_sb[:, ci:ci + 1],
                                )
                            else:
                                nc.vector.tensor_scalar(
                                    dst, src, b1_sb[:, ci:ci + 1], 0.0,
                                    AL.add, AL.max,
                                )

                    def conv1_chunk(k):
                        for lp in range(CHUNK // 2):
                            conv1_pair(k, lp)
                        pas.pop(k)

                    def conv2_group(co_blk, g, g0, nb, w2cs, interleave=()):
                        """One conv2 output group; optionally interleaves
                        pending conv1 pair thunks into the matmul stream so
                        their epilogues drain during conv2's long PSUM
                        accumulation (no conv1 stalls)."""
                        interleave = list(interleave)
                        stride = max(1, (162 // (len(interleave) + 1))
                                     if interleave else 162)
                        ps2 = ps2p.tile([128, 504], F32, tag="ps2", bufs=2)
                        pstep = ps2.ap[0][0]
                        out4 = _r(ps2, [[pstep, 128], [36, nb], [6, 6], [1, 6]])
                        n_mm = 0
                        for ci in range(2):
                            h1 = h1s[ci]
                            hp = h1.ap[0][0]
                            for kk in range(81):
                                ky, kx = divmod(kk, 9)
                                rhs = AP(h1.tensor,
                                         h1.offset + g0 * 400 + ky * 20 + kx,
                                         [[hp, 128], [400, nb], [40, 6], [2, 6]])
                                nc.tensor.matmul(
                                    out4,
                                    w2cs[ci][:, kk * 128:(kk + 1) * 128],
                                    rhs,
                                    start=(ci == 0 and kk == 0),
                                    stop=(ci == 1 and kk == 80),
                                )
                                n_mm += 1
                                if interleave and n_mm % stride == 0:
                                    interleave.pop(0)()
                        for t in interleave:
                            t()
                        # bias+relu, write pix-major (col = pix*BC + b)
                        xr = xr_sb[co_blk]
                        nc.scalar.activation(
                            AP(xr.tensor, xr.offset + g0,
                               [[xr.ap[0][0], 128], [1, nb], [BC, 36]]),
                            _r(ps2, [[pstep, 128], [36, nb], [1, 36]]),
                            AF.Relu,
                            bias=pcb_sb[:, co_blk:co_blk + 1],
                        )

                    def warmup_cc(dep_ap):
                        """Collective keyed on conv progress: absorbs
                        inter-core skew off the critical path so the real
                        AllReduces are fast."""
                        nonlocal n_warm
                        cinw = dpool.tile([128, 1], F32, name=f"cinw{n_warm}")
                        coutw = dpool.tile([128, 1], F32, name=f"coutw{n_warm}",
                                           addr_space="Shared")
                        n_warm += 1
                        nc.gpsimd.dma_start(cinw[:], dep_ap)
                        nc.gpsimd.collective_compute(
                            "AllReduce", AL.add,
                            replica_groups=[list(range(NCORES))],
                            ins=[cinw.opt()], outs=[coutw.opt()],
                        )

                    # group-pipelined conv: conv1 chunks feed conv2 co0
                    # groups, with later chunks' image-pairs interleaved INTO
                    # the conv2 matmul stream. ALL bulk DMAs ride the sync
                    # queue so the scalar engine stays free for epilogues.
                    load_chunk(0, nc.sync)
                    load_chunk(1, nc.sync)
                    w2c0 = [load_w2c(0, 0, nc.sync)]
                    load_chunk(2, nc.sync)
                    conv1_chunk(0)
                    load_chunk(3, nc.sync)
                    conv1_chunk(1)
                    w2c0.append(load_w2c(0, 1, nc.sync))
                    # small routing constants, nothing urgent
                    nc.sync.dma_start(eye_sb[:], eye64[:])
                    nc.sync.dma_start(maskT_sb[:], maskT[:])
                    nc.sync.dma_start(mask32_sb[:], mask32[:])
                    for cb in range(2):
                        nc.sync.dma_start(
                            w2p_sb[cb][:],
                            AP(w2p[:].tensor, cb * 128 * 36 * HL,
                               [[36 * HL, 128], [1, 36 * HL]]),
                        )
                    load_chunk(4, nc.sync)

                    def pairs(k, then_load=None):
                        out = []
                        for lp in range(CHUNK // 2):
                            def thunk(k=k, lp=lp, last=(lp == CHUNK // 2 - 1)):
                                conv1_pair(k, lp)
                                if last:
                                    pas.pop(k)
                                    if then_load is not None:
                                        load_chunk(then_load, nc.sync)
                            out.append(thunk)
                        return out

                    ilv = {0: pairs(2, 5) + pairs(3, 6),
                           1: pairs(4, 7) + pairs(5),
                           2: pairs(6), 3: pairs(7), 4: []}
                    for g, (g0, nb) in enumerate(GROUPS):
                        conv2_group(0, g, g0, nb, w2c0, ilv[g])
                        if g == 0:
                            warmup_cc(xr_sb[0][:, 0:1])
                    # round-1 s-matmul, co0 half: c is uniform in round 1, so
                    # these fold into the conv stream as soon as xr0 is done
                    s_ps1 = sps.tile([BC, HL], F32, tag="s_ps")
                    for pix in range(36):
                        nc.tensor.matmul(
                            s_ps1[:],
                            xr_sb[0][:, pix * BC:(pix + 1) * BC],
                            w2p_sb[0][:, pix * HL:(pix + 1) * HL],
                            start=(pix == 0), stop=False,
                        )
                    w2c1 = [load_w2c(1, 0, nc.sync), load_w2c(1, 1, nc.scalar)]
                    for g, (g0, nb) in enumerate(GROUPS):
                        conv2_group(1, g, g0, nb, w2c1)
                        if g == 0:
                            warmup_cc(xr_sb[1][:, 0:1])

            # ---------------- routing phase ----------------
            with (
                tc.tile_pool(name="rsb", bufs=1) as rsb,
                tc.tile_pool(name="rnd", bufs=2) as rnd,
                tc.tile_pool(name="gps", bufs=4, space="PSUM") as gps,
                tc.tile_pool(name="zps", bufs=1, space="PSUM") as zps,
            ):
                # W2^T for the P matmuls: loaded at routing start (h1 freed)
                w2pt_a_sb = [rsb.tile([128, 36 * 128], F16, tag=f"w2pta{cb}",
                                      name=f"w2pta{cb}") for cb in range(2)]
                for cb in range(2):
                    (nc.sync if cb == 0 else nc.scalar).dma_start(
                        w2pt_a_sb[cb][:],
                        AP(w2pt_a[:].tensor, cb * 128 * 36 * 128,
                           [[36 * 128, 128], [1, 36 * 128]]),
                    )
                w2ptb2_sb = [rsb.tile([64, 18 * 128], F16, tag=f"w2ptb{cb}",
                                      name=f"w2ptb{cb}") for cb in range(2)]
                for cb in range(2):
                    (nc.sync if cb == 0 else nc.scalar).dma_start(
                        w2ptb2_sb[cb][:],
                        AP(w2ptb2[:].tensor, cb * 64 * 18 * 128,
                           [[18 * 128, 64], [1, 18 * 128]]),
                    )
                prod = rsb.tile([128, 2 * 36 * BC], F16)

                def s_matmul():
                    s_ps = sps.tile([BC, HL], F32, tag="s_ps")
                    first, last = (0, 0), (1, 35)
                    for cb in range(2):
                        for pix in range(36):
                            nc.tensor.matmul(
                                s_ps[:],
                                xr_sb[cb][:, pix * BC:(pix + 1) * BC],
                                w2p_sb[cb][:, pix * HL:(pix + 1) * HL],
                                start=((cb, pix) == first),
                                stop=((cb, pix) == last),
                            )
                    return s_ps

                def squash(s_sb, out_dtype):
                    sq = rnd.tile([BC, HL], F32, tag="sq")
                    nc.scalar.square(sq[:], s_sb[:])
                    n2 = rnd.tile([BC, 16], F32, tag="n2")
                    nc.vector.tensor_reduce(
                        n2[:].rearrange("a b -> a b ()"),
                        _r(sq, [[sq.ap[0][0], BC], [1, 16], [16, 10]]),
                        AX.X, AL.add,
                    )
                    rt = rnd.tile([BC, 16], F32, tag="rt")
                    nc.scalar.sqrt(rt[:], n2[:])
                    n2p1 = rnd.tile([BC, 16], F32, tag="n2p1")
                    nc.vector.tensor_scalar_add(n2p1[:], n2[:], 1.0)
                    rcp = rnd.tile([BC, 16], F32, tag="rcp")
                    nc.vector.reciprocal(rcp[:], n2p1[:])
                    f = rnd.tile([BC, 16], F32, tag="f")
                    nc.vector.tensor_tensor(f[:], rt[:], rcp[:], AL.mult)
                    v_sb = rnd.tile([BC, HL], out_dtype, tag="v_sb")
                    nc.vector.tensor_tensor(
                        _r(v_sb, [[v_sb.ap[0][0], BC], [16, 10], [1, 16]]),
                        _r(s_sb, [[s_sb.ap[0][0], BC], [16, 10], [1, 16]]),
                        _r(f, [[f.ap[0][0], BC], [0, 10], [1, 16]]),
                        AL.mult,
                    )
                    return v_sb

                def p_delta_update(v16, rnd_idx, rce32):
                    """delta via P[co,pix,b] = sum_hl W2*v (PE), then
                    D[co,pix] = sum_b xr*P (DVE), then delta32[c32,pix] =
                    mask-matmul partition regroup. If xr is c-scaled, divide
                    by ce32 (rce32 ap) to undo."""
                    # vt_ps cols: [0:BC) = v[:, :128]^T; [BC:3*BC) = the
                    # block-diagonal diag2(v[:, 128:160]^T) used by the packed
                    # P_b matmul (one K=64 matmul covers the hl-tail of TWO
                    # pixels at once)
                    vt_ps = gps.tile([128, 3 * BC], F16, tag="vt_ps", bufs=1)
                    nc.tensor.transpose(vt_ps[:, 0:BC], v16[:, 0:128], eye_sb[:])
                    for q in range(2):
                        nc.tensor.transpose(
                            AP(vt_ps.tensor,
                               vt_ps.offset + q * 32 * vt_ps.ap[0][0]
                               + (1 + q) * BC,
                               [[vt_ps.ap[0][0], 32], [1, BC]]),
                            v16[:, 128:160], eye_sb[:])
                    vt_a = rnd.tile([128, BC], F16, tag="vt_a")
                    nc.scalar.copy(vt_a[:], vt_ps[:, 0:BC])
                    for q in range(2):
                        nc.scalar.copy(
                            AP(vb2.tensor,
                               vb2.offset + q * 32 * vb2.ap[0][0] + q * BC,
                               [[vb2.ap[0][0], 32], [1, BC]]),
                            AP(vt_ps.tensor,
                               vt_ps.offset + q * 32 * vt_ps.ap[0][0]
                               + (1 + q) * BC,
                               [[vt_ps.ap[0][0], 32], [1, BC]]))
                    # P in 4-pix batches; DVE multiplies straight out of PSUM
                    TB = 4
                    for cb in range(2):
                        for pb in range(36 // TB):
                            p_ps = gps.tile([128, TB * BC], F32, tag="p_ps",
                                            bufs=2)
                            for j in range(TB):
                                pix = pb * TB + j
                                nc.tensor.matmul(
                                    p_ps[:, j * BC:(j + 1) * BC],
                                    w2pt_a_sb[cb][:, pix * 128:(pix + 1) * 128],
                                    vt_a[:],
                                    start=True, stop=False,
                                )
                            for j2 in range(2):
                                pp2 = pb * 2 + j2
                                nc.tensor.matmul(
                                    p_ps[:, j2 * 2 * BC:(j2 + 1) * 2 * BC],
                                    w2ptb2_sb[cb][:, pp2 * 128:(pp2 + 1) * 128],
                                    vb2[:],
                                    start=False, stop=True,
                                )
                            xh = xr_sb[cb]
                            nc.vector.tensor_tensor(
                                prod[:, (cb * 36 + pb * TB) * BC:
                                     (cb * 36 + pb * TB + TB) * BC],
                                AP(xh.tensor, xh.offset + pb * TB * BC,
                                   [[xh.ap[0][0], 128], [1, TB * BC]]),
                                p_ps[:],
                                AL.mult,
                            )
                    ds_ps = gps.tile([32, 36], F32, tag="ds_ps", bufs=1)
                    for cb in range(2):
                        D = rnd.tile([128, 36], F32, tag=f"D{cb}")
                        nc.vector.tensor_reduce(
                            D[:].rearrange("a b -> a b ()"),
                            AP(prod.tensor, prod.offset + cb * 36 * BC,
                               [[prod.ap[0][0], 128], [BC, 36], [1, BC]]),
                            AX.X, AL.add,
                        )
                        # regroup: delta32[c32,pix] = sum_{p: p%32==c32} D[p,pix]
                        nc.tensor.matmul(
                            ds_ps[:], maskT_sb[:], D[:],
                            start=(cb == 0), stop=(cb == 1),
                        )
                    delta32 = rnd.tile([32, 36], F32, tag="delta32")
                    if rce32 is not None:
                        nc.vector.tensor_tensor(
                            delta32[:], ds_ps[:], rce32[:], AL.mult)
                    else:
                        nc.scalar.copy(delta32[:], ds_ps[:])
                    cin = dpool.tile([32, 36], F32, name=f"cin{rnd_idx}")
                    cout = dpool.tile([32, 36], F32, name=f"cout{rnd_idx}",
                                      addr_space="Shared")
                    nc.gpsimd.dma_start(cin[:], delta32[:])
                    nc.gpsimd.collective_compute(
                        "AllReduce", AL.add,
                        replica_groups=[list(range(NCORES))],
                        ins=[cin.opt()], outs=[cout.opt()],
                    )
                    dsum = rnd.tile([32, 36], F32, tag="dsum")
                    nc.gpsimd.dma_start(dsum[:], cout[:])
                    if rnd_idx == 0:
                        nc.scalar.mul(b32[:], dsum[:], ROUTE_SCALE)
                    else:
                        sc = rnd.tile([32, 36], F32, tag="sc")
                        nc.scalar.mul(sc[:], dsum[:], ROUTE_SCALE)
                        nc.vector.tensor_tensor(b32[:], b32[:], sc[:], AL.add)

                def softmax_ce():
                    """ce32[c32,pix] = softmax(b32)[n=c32*36+pix], F32."""
                    e32 = rnd.tile([32, 36], F32, tag="e32")
                    rs = rnd.tile([32, 1], F32, tag="rs")
                    nc.scalar.activation(e32[:], b32[:], AF.Exp,
                                         accum_out=rs[:])
                    z_ps = zps.tile([1, 1], F32, tag="z_ps")
                    nc.tensor.matmul(z_ps[:], ones32[:], rs[:], start=True, stop=True)
                    z_sb = rnd.tile([1, 1], F32, tag="z_sb")
                    nc.scalar.copy(z_sb[:], z_ps[:])
                    zb_ps = zps.tile([32, 1], F32, tag="zb_ps")
                    nc.tensor.matmul(zb_ps[:], ones1[:], z_sb[:], start=True, stop=True)
                    rz = rnd.tile([32, 1], F32, tag="rz")
                    nc.vector.reciprocal(rz[:], zb_ps[:])
                    ce32 = rnd.tile([32, 36], F32, tag="ce32")
                    nc.vector.tensor_scalar_mul(ce32[:], e32[:], rz[:])
                    return ce32

                def scale_xr(m32f32):
                    """xr[co, pix, b] *= m32[co%32, pix] in place."""
                    cm_ps = zps.tile([128, 36], F32, tag="cm_ps")
                    nc.tensor.matmul(cm_ps[:], mask32_sb[:], m32f32[:],
                                     start=True, stop=True)
                    for cb in range(2):
                        xh = xr_sb[cb]
                        nc.vector.tensor_tensor(
                            _r(xh, [[xh.ap[0][0], 128], [BC, 36], [1, BC]]),
                            _r(xh, [[xh.ap[0][0], 128], [BC, 36], [1, BC]]),
                            _r(cm_ps, [[cm_ps.ap[0][0], 128], [1, 36], [0, BC]]),
                            AL.mult,
                        )

                # ---- round 1 (c uniform; xr unscaled) ----
                # co0 half was accumulated into s_ps1 during the conv phase
                for pix in range(36):
                    nc.tensor.matmul(
                        s_ps1[:],
                        xr_sb[1][:, pix * BC:(pix + 1) * BC],
                        w2p_sb[1][:, pix * HL:(pix + 1) * HL],
                        start=False, stop=(pix == 35),
                    )
                s_sb = rnd.tile([BC, HL], F32, tag="s_sb")
                nc.scalar.mul(s_sb[:], s_ps1[:], 1.0 / 1152.0)
                v16 = squash(s_sb, F16)
                p_delta_update(v16, 0, None)
                # ---- round 2 ----
                ce2 = softmax_ce()
                scale_xr(ce2)
                s_ps = s_matmul()
                # off the critical path: runs on DVE while the PE streams s
                rce32 = rnd.tile([32, 36], F32, tag="rce32")
                nc.vector.reciprocal(rce32[:], ce2[:])
                s_sb = rnd.tile([BC, HL], F32, tag="s_sb")
                nc.scalar.copy(s_sb[:], s_ps[:])
                v16 = squash(s_sb, F16)
                p_delta_update(v16, 1, rce32)
                # ---- round 3 (b update dead) ----
                ce3 = softmax_ce()
                ratio32 = rnd.tile([32, 36], F32, tag="ratio32")
                nc.vector.tensor_tensor(ratio32[:], ce3[:], rce32[:], AL.mult)
                scale_xr(ratio32)
                s_ps = s_matmul()
                s_sb = rnd.tile([BC, HL], F32, tag="s_sb")
                nc.scalar.copy(s_sb[:], s_ps[:])
                v_sb = squash(s_sb, F32)
                nc.sync.dma_start(vout[:], v_sb[:])

    return nc


_NC_CACHE = None


def _get_nc():
    global _NC_CACHE
    if _NC_CACHE is None:
        nc = build_nc()
        split_waits(nc)
        _NC_CACHE = nc
    return _NC_CACHE


def prepare_inputs(x, conv1_w, conv1_b, pc_w, pc_b, W):
    x = np.asarray(x, np.float32)
    xf = np.zeros((B, 800), np.float16)
    xf[:, :784] = x.reshape(B, 784).astype(np.float16)
    # host-side im2col ("wide patch"): xp[i, (ky,kx), j] = xf[i, 28*ky+kx+j]
    xp = np.lib.stride_tricks.as_strided(
        xf, shape=(B, 9, 9, 560), strides=(1600, 56, 2, 2)).reshape(B, 81, 560)
    w1t = np.ascontiguousarray(
        np.asarray(conv1_w, np.float32).reshape(256, 81).T).astype(np.float16)
    b1 = np.ascontiguousarray(np.asarray(conv1_b, np.float32))
    # pcw4[co_blk, ci_blk, ci128, kk*128co] = pc_w[co, ci, ky, kx]
    pcw = np.asarray(pc_w, np.float32).reshape(256, 256, 81)  # [co, ci, kk]
    pcw4 = np.ascontiguousarray(
        pcw.reshape(2, 128, 2, 128, 81).transpose(0, 2, 3, 4, 1)
    ).astype(np.float16)  # [co_blk, ci_blk, ci128, kk, co128]
    pcb = np.ascontiguousarray(np.asarray(pc_b, np.float32).reshape(256))
    # W2cp[co, pix, hl] = W2n[co*36+pix, hl]
    w2n = np.asarray(W, np.float32).transpose(3, 0, 1, 2).reshape(9216, HL)
    w2cp = w2n.reshape(256, 36, HL)
    w2p = np.ascontiguousarray(
        w2cp.reshape(2, 128, 36 * HL)).astype(np.float16)
    w2t = w2cp.transpose(2, 1, 0)                 # [hl, pix, co]
    w2pt_a = np.ascontiguousarray(
        w2t[:128].reshape(128, 36, 2, 128).transpose(2, 0, 1, 3)
        .reshape(2, 128, 36 * 128)).astype(np.float16)
    # w2ptb2[cb][(q, hl'), pp2*128+co] = w2t[128+hl', pp2*2+q, cb*128+co]
    w2ptb2 = np.ascontiguousarray(
        w2t[128:].reshape(32, 18, 2, 2, 128).transpose(3, 2, 0, 1, 4)
        .reshape(2, 64, 18 * 128)).astype(np.float16)
    eye64 = np.eye(BC, dtype=np.float16)
    maskT = np.zeros((128, 32), np.float32)
    maskT[np.arange(128), np.arange(128) % 32] = 1.0
    mask32 = np.ascontiguousarray(maskT.T)
    in_maps = []
    for c in range(NCORES):
        in_maps.append({
            "xpatch": np.ascontiguousarray(
                xp[c * BC:(c + 1) * BC].transpose(1, 0, 2).reshape(81, BC * 560)),
            "w1t": w1t, "b1": b1, "pcw4": pcw4, "pcb": pcb, "w2p": w2p,
            "w2pt_a": w2pt_a, "w2ptb2": w2ptb2, "eye64": eye64,
            "maskT": maskT, "mask32": mask32,
        })
    return in_maps


def kernel(x, conv1_w, conv1_b, pc_w, pc_b, W, _trace=False, _trace_kwargs=None):
    nc = _get_nc()
    in_maps = prepare_inputs(x, conv1_w, conv1_b, pc_w, pc_b, W)
    res = run_bass_kernel_spmd(
        nc, in_maps, list(range(NCORES)),
        trace=_trace, **(_trace_kwargs or {}),
    )
    v = np.concatenate([np.asarray(res.results[c]["vout"]) for c in range(NCORES)], 0)
    out = v.reshape(B, 1, 1, 10, 16).astype(np.float32)
    if _trace:
        return out, res
    return out
